# revision 32
# baseline (speedup 1.0000x reference)
"""Trainium2 Bass kernel for causal multi-head attention (software-pipelined).

Problem: B=4, S=2048, D=1024, H=16 heads (d_head=64), fp32 I/O.
    qkv = x @ w_qkv + b_qkv ; causal softmax attention ; out @ w_out + b_out

Sharding over 8 NeuronCores: data-parallel over batch (4) x tensor-parallel
over head-groups (2 groups of 8 heads). Core c handles batch c//2, head-group
c%2. No collectives: each core returns its partial out-projection
y_partial = attn_out_g @ w_out[rows_g]; the host sums the two group partials
per batch and adds b_out (plus b_v @ w_out -- see below).

Design notes (v2, evolved from the 311us phase-woven baseline):
  - 512-wide query blocks b=0..3; key j-tiles of 128; head pairs p=0..3
    (heads 2p, 2p+1 live in partition halves of qT/kT tiles; the two scores
    matmuls per j-tile are 64x128 row-tiles T0/T8 that stream CONCURRENTLY).
  - j-tiles processed in GROUPS OF TWO per PE mode: [scores jt0 + jt1]
    (64x128 tiling mode) then [AV jt0' + jt1' of the previous group]
    (128x128 mode).  Mode switches drain the PE array (~110ns each);
    grouping halves them vs per-j-tile alternation (trace: scores pair
    317ns vs 216 roofline = unhidden LDW + drain; AV s0 336 vs 216).
  - The two packed heads write ONE merged scores PSUM tile [128, 2, 512]
    (2 banks), one ScalarE exp covers both heads per j-tile.
  - AV accumulates [65, 512] per sub with a ones column in v producing the
    softmax denominator; max-free softmax (logits < ~7).
  - v-projection bias is folded OUT of the device: sum(attn)=1 makes
    av_norm(v_raw + bv) = av_norm(v_raw) + bv, and bv passes through the
    out-projection as the constant row bv @ w_out added host-side into
    b_out.  This deletes 16 K=1 N=512 bias matmuls (~5us PE).
  - Projections (qk via stationary w, v natural) and the out-projection are
    split into small units woven between attention groups from a generator
    queue; deps are prefetched TWO sections ahead so block transitions
    (2 qk + 4 v units = ~13us of PE) never starve the weave.
  - qk bias+PSUM-drain runs on VectorE (tensor_scalar_add), ScalarE stays
    exp-only.
  - PSUM budget: scores 2x2 + av 2x1 + proj/out 2x1 = 8 banks exactly.
  - DMA: inputs split across both HWDGE queues (Sync: wq + x[:, :512] the
    lead needs, then x tails + wo; Scalar: small consts + wk + wv).
    bq/bk packed host-side into one [128, 8] tensor.  A dummy exp preloads
    the ACT table during the DMA lead; a dependency-free 36-matmul warm-up
    block on a MEMSET tile (no DMA gate) un-throttles the HAM clock gate
    ~7us earlier than the tri-gated variant.
  - Output y is stored bf16 (halves the 8MB/core store drain; host sums
    the two partials in f32; adds ~0.1% rounding, budget is 2e-2).
  - Tail: the last block's first two out-projection chunks split into a
    k<=2 partial woven during the final section + a k=3 tail after the
    final norms (accumulation-group semaphore waits hoist to the group
    head and would otherwise serialize behind the final normalization).
"""

import sys

if "/opt/trn_rl_repo" not in sys.path:
    sys.path.insert(0, "/opt/trn_rl_repo")

from collections import deque

import numpy as np
import ml_dtypes

B, S, D = 4, 2048, 1024
H, DH = 16, 64
G = 2                # tensor-parallel head groups
HPG = H // G         # heads per group (8)
CG = HPG * DH        # channel cols per group (512)
N_CORES = 8
BF16 = ml_dtypes.bfloat16

KT = D // 128        # 8 contraction k-tiles for the projections
QB = 512             # query block width
NB = S // QB         # 4 query blocks

_cache = {}


def _build_program():
    import concourse.tile as tile
    from concourse import bacc, mybir

    f32 = mybir.dt.float32
    bf16 = mybir.dt.bfloat16
    Exp = mybir.ActivationFunctionType.Exp
    Copy = mybir.ActivationFunctionType.Copy
    SCALE = float(DH) ** -0.5

    nc = bacc.Bacc("TRN2", target_bir_lowering=False, debug=False,
                   num_devices=N_CORES)

    xT_d = nc.dram_tensor("xT", [D, S], bf16, kind="ExternalInput").ap()
    wq_d = nc.dram_tensor("wq", [D, CG], bf16, kind="ExternalInput").ap()
    wk_d = nc.dram_tensor("wk", [D, CG], bf16, kind="ExternalInput").ap()
    wv_d = nc.dram_tensor("wv", [D, CG], bf16, kind="ExternalInput").ap()
    # bq/bk packed as one [128, 8] tensor (col m = bq chunk m, col 4+m = bk
    # chunk m): a single DMA instead of eight 4-byte-element ones
    bqk_d = nc.dram_tensor("bqk", [128, 8], f32, kind="ExternalInput").ap()
    wo_d = nc.dram_tensor("wo", [CG, D], bf16, kind="ExternalInput").ap()
    tri_d = nc.dram_tensor("tri", [128, 128], bf16, kind="ExternalInput").ap()
    y_d = nc.dram_tensor("y", [S, D], bf16, kind="ExternalOutput").ap()
    # k=3 tail contributions of the last query block (host adds to y rows
    # 1536:2048 -- lets the post-final-norms eviction run on the idle
    # ScalarE instead of the busy VectorE)
    y2_d = nc.dram_tensor("y2", [QB, D], bf16, kind="ExternalOutput").ap()

    with tile.TileContext(nc) as tc:
        with (
            tc.tile_pool(name="consts", bufs=1) as cpool,
            tc.tile_pool(name="acts", bufs=1) as apool,
            tc.tile_pool(name="exps", bufs=6) as epool,
            tc.tile_pool(name="small", bufs=2) as spool,
            tc.tile_pool(name="rbc", bufs=2) as rpool,
            tc.tile_pool(name="ystage", bufs=3) as ypool,
            tc.tile_pool(name="psum_s", bufs=2, space="PSUM") as sp,
            tc.tile_pool(name="psum_av", bufs=2, space="PSUM") as avp,
            tc.tile_pool(name="psum_p", bufs=2, space="PSUM") as pp,
        ):
            # ---- DMA loads, split across the two HWDGE queues (Sync +
            # Scalar) to double input bandwidth. ----
            ones_row = cpool.tile([1, 128], bf16, tag="ones")
            nc.gpsimd.memset(ones_row[:], 1.0)
            # warm-up stationary: memset (NOT DMA-gated) so the HAM warm-up
            # can start during the bass preamble
            wrm = cpool.tile([128, 128], bf16, tag="wrm")
            nc.gpsimd.memset(wrm[:], 0.25)
            # dummy exp on a const tile: forces the ACT table load (~2.7us)
            # to happen during the DMA lead, before the ScalarE-queue DMAs
            warm_exp = spool.tile([1, 128], f32, tag="wexp", name="warm_exp")
            nc.scalar.activation(warm_exp[:], ones_row[:], Exp, scale=1.0)

            bqk = cpool.tile([128, 8], f32, tag="bqk")
            nc.scalar.dma_start(bqk[:], bqk_d[:])
            bqc = [bqk[:, m:m + 1] for m in range(4)]
            bkc = [bqk[:, 4 + m:5 + m] for m in range(4)]
            tri = cpool.tile([128, 128], bf16, tag="tri")
            nc.scalar.dma_start(tri[:], tri_d[:])

            wq, wk, wv, xt = [], [], [], []
            for k in range(KT):
                rows = slice(k * 128, (k + 1) * 128)
                t = cpool.tile([128, CG], bf16, tag=f"wq{k}", name=f"wq{k}")
                nc.sync.dma_start(t[:], wq_d[rows, :])
                wq.append(t)
                t = cpool.tile([128, CG], bf16, tag=f"wk{k}", name=f"wk{k}")
                nc.scalar.dma_start(t[:], wk_d[rows, :])
                wk.append(t)
                t = cpool.tile([128, CG], bf16, tag=f"wv{k}", name=f"wv{k}")
                nc.scalar.dma_start(t[:], wv_d[rows, :])
                wv.append(t)
                t = cpool.tile([128, S], bf16, tag=f"xt{k}", name=f"xt{k}")
                nc.sync.dma_start(t[:, 0:QB], xT_d[rows, 0:QB])
                xt.append(t)
            for k in range(KT):
                nc.sync.dma_start(xt[k][:, QB:S],
                                  xT_d[k * 128:(k + 1) * 128, QB:S])
            wo = []
            for k in range(CG // 128):
                t = cpool.tile([128, D], bf16, tag=f"wo{k}", name=f"wo{k}")
                nc.sync.dma_start(t[:], wo_d[k * 128:(k + 1) * 128, :])
                wo.append(t)

            # ---- persistent activations ----
            qT = [apool.tile([128, S], bf16, tag=f"qT{m}", name=f"qT{m}")
                  for m in range(CG // 128)]
            kTt = [apool.tile([128, S], bf16, tag=f"kT{m}", name=f"kT{m}")
                   for m in range(CG // 128)]
            # v with a ones column per head: [s, h, 0:64] = v_h, [s, h, 64] = 1
            vst = [apool.tile([128, HPG, DH + 1], bf16, tag=f"v{st}",
                              name=f"v{st}")
                   for st in range(S // 128)]
            aoT = [apool.tile([128, S], bf16, tag=f"aoT{m}", name=f"aoT{m}")
                   for m in range(CG // 128)]

            for st in range(S // 128):
                nc.gpsimd.memset(vst[st][:, :, DH:DH + 1], 1.0)

            # ---- filler units (generators yielding between PE matmuls) ----
            def qk_unit(m, n, acc=None):
                wt, bc, dst = (wq, bqc, qT) if m < 4 else (wk, bkc, kTt)
                mi = m % 4
                if acc is None:
                    acc = pp.tile([128, QB], f32, tag="pp", name=f"qk{m}_{n}")
                for k in range(KT):
                    nc.tensor.matmul(
                        acc[:], wt[k][:, mi * 128:(mi + 1) * 128],
                        xt[k][:, n * QB:(n + 1) * QB],
                        start=(k == 0), stop=(k == KT - 1))
                    yield
                nc.vector.tensor_scalar_add(
                    dst[mi][:, n * QB:(n + 1) * QB], acc[:], bc[mi])
                yield

            def v_unit(st, acc=None):
                if acc is None:
                    acc = pp.tile([128, HPG, DH], f32, tag="pp",
                                  name=f"vacc{st}")
                for k in range(KT):
                    nc.tensor.matmul(
                        acc[:, :, :], xt[k][:, st * 128:(st + 1) * 128],
                        wv[k][:], start=(k == 0), stop=(k == KT - 1))
                    yield
                nc.vector.tensor_copy(vst[st][:, :, 0:DH], acc[:, :, :])
                yield

            def out_unit(st, n):
                yp = pp.tile([128, 512], f32, tag="pp", name=f"yp{st}_{n}")
                for k in range(CG // 128):
                    nc.tensor.matmul(
                        yp[:], aoT[k][:, st * 128:(st + 1) * 128],
                        wo[k][:, n * 512:(n + 1) * 512],
                        start=(k == 0), stop=(k == CG // 128 - 1))
                    yield
                ys = ypool.tile([128, 512], bf16, tag="ys", name=f"ys{st}_{n}")
                nc.vector.tensor_copy(ys[:], yp[:])
                nc.sync.dma_start(
                    y_d[st * 128:(st + 1) * 128, n * 512:(n + 1) * 512], ys[:])
                yield

            # split variants for the LAST block: the k=0..2 partial (deps:
            # head pairs 0..2 only) weaves during the final section and is
            # EVICTED to SBUF bf16, freeing its PSUM box so all 8 chunks can
            # pre-run; the k=3 matmul + add + store runs after the final
            # norm.  Costs one extra bf16 rounding on 3/4 of y (~0.1%).
            def out_head(st, n):
                yp = pp.tile([128, 512], f32, tag="pp", name=f"yph{st}_{n}")
                for k in range(3):
                    nc.tensor.matmul(
                        yp[:], aoT[k][:, st * 128:(st + 1) * 128],
                        wo[k][:, n * 512:(n + 1) * 512],
                        start=(k == 0), stop=(k == 2))
                    yield
                part = ypool.tile([128, 512], bf16, tag="ys",
                                  name=f"part{st}_{n}")
                nc.vector.tensor_copy(part[:], yp[:])
                nc.sync.dma_start(
                    y_d[st * 128:(st + 1) * 128, n * 512:(n + 1) * 512],
                    part[:])
                yield

            def out_tail(st, n, yp, eng):
                # yp: a dedicated PSUM view (scores banks are free after the
                # final exps) so all 8 tail matmuls issue back-to-back; the
                # evictions alternate between the (exp-done, idle) ScalarE
                # and VectorE into y2, which the host adds to y rows
                # 1536:2048.
                nc.tensor.matmul(
                    yp[:], aoT[3][:, st * 128:(st + 1) * 128],
                    wo[3][:, n * 512:(n + 1) * 512],
                    start=True, stop=True)
                yield
                ys = ypool.tile([128, 512], bf16, tag="ys2", bufs=8,
                                name=f"ys2_{st}_{n}")
                if eng == 0:
                    nc.scalar.activation(ys[:], yp[:], Copy, scale=1.0)
                else:
                    nc.vector.tensor_copy(ys[:], yp[:])
                nc.sync.dma_start(
                    y2_d[(st - 12) * 128:(st - 11) * 128,
                         n * 512:(n + 1) * 512], ys[:])
                yield

            def warm_unit():
                # ~4us of dependency-free full-array matmuls: fills the
                # preamble/DMA lead and un-throttles the HAM clock gate
                # (which watches PE array activity) before dense work.
                wp = pp.tile([128, 128], f32, tag="pp", name="warmps")
                for i in range(36):
                    nc.tensor.matmul(wp[:], wrm[:], wrm[:],
                                     start=True, stop=True)
                    yield

            class Unit:
                __slots__ = ("gen", "done")

                def __init__(self, gen):
                    self.gen = gen
                    self.done = False

                def step(self):
                    if self.done:
                        return False
                    try:
                        next(self.gen)
                        return True
                    except StopIteration:
                        self.done = True
                        return False

            fill_q = deque()

            def weave(n):
                while n > 0 and fill_q:
                    u = fill_q[0]
                    if u.step():
                        n -= 1
                    else:
                        fill_q.popleft()

            def force(units):
                while not all(u.done for u in units):
                    weave(1)
                    if not fill_q:
                        break

            # ---- attention section for (head pair p, query block b) ----
            def norms(avs, p, b):
                # normalize by the broadcast fast-reciprocal of the ones-row
                # denominator; custom-DVE recip must not read PSUM: stage the
                # row in SBUF first.  The two subs' chains are interleaved so
                # the gpsimd broadcasts overlap the DVE ops.
                dn, rc, rb = [None, None], [None, None], [None, None]
                for s in (0, 1):
                    dn[s] = spool.tile([1, QB], f32, tag=f"dn{s}", name="dn")
                    nc.vector.tensor_copy(dn[s][:], avs[s][DH:DH + 1, :])
                    rc[s] = spool.tile([1, QB], f32, tag=f"rc{s}", name="rc")
                    nc.vector.reciprocal_approx_fast(rc[s][:], dn[s][:])
                for s in (0, 1):
                    rb[s] = rpool.tile([DH, QB], f32, tag=f"rb{s}", name="rb")
                    nc.gpsimd.partition_broadcast(rb[s][:], rc[s][:])
                if not (p == 3 and b == 3):
                    for s in (0, 1):
                        nc.vector.tensor_mul(
                            aoT[p][DH * s:DH * s + DH, b * QB:(b + 1) * QB],
                            avs[s][0:DH, :], rb[s][:])
                else:
                    # final section: per 128-query chunk, st-ascending, so
                    # the out-projection k=3 tails unblock progressively
                    for c in range(4):
                        cs = slice(c * 128, (c + 1) * 128)
                        for s in (0, 1):
                            nc.vector.tensor_mul(
                                aoT[p][DH * s:DH * s + DH,
                                       b * QB + c * 128:
                                       b * QB + (c + 1) * 128],
                                avs[s][0:DH, cs], rb[s][:, cs])

            def av_step(p, njt, avs, jt, et, c0):
                for s in (0, 1):
                    nc.tensor.matmul(
                        avs[s][:, c0:QB], vst[jt][:, 2 * p + s, :],
                        et[:, s, c0:QB],
                        start=(jt == 0), stop=(jt == njt - 1))

            WEAVE = (8, 6, 4, 4)   # filler steps per group, by block

            def section(p, b):
                njt = 4 * (b + 1)
                avs = [avp.tile([DH + 1, QB], f32, tag="av",
                                name=f"av{p}_{b}_{s}") for s in (0, 1)]
                prev = []
                for g in range(njt // 2):
                    # scores for both j-tiles of the group (64x128 tiling
                    # mode, pairs stream concurrently; 2nd pair's LDW hides
                    # behind the 1st pair's matmuls)
                    cur = []
                    for jt in (2 * g, 2 * g + 1):
                        off = jt - 4 * b
                        c0 = 128 * off if off > 0 else 0
                        ps = sp.tile([128, 2, QB], f32, tag="ps",
                                     name=f"ps{p}_{b}_{jt}")
                        for s in (0, 1):
                            nc.tensor.matmul(
                                ps[:, s, c0:QB],
                                kTt[p][DH * s:DH * s + DH,
                                       jt * 128:(jt + 1) * 128],
                                qT[p][DH * s:DH * s + DH,
                                      b * QB + c0:(b + 1) * QB],
                                start=True, stop=True)
                        cur.append((jt, ps, c0, off))
                    # previous group's AV pairs (128x128 mode)
                    for jt, et, c0 in prev:
                        av_step(p, njt, avs, jt, et, c0)
                    # exps (ScalarE) + diagonal masks (VectorE) for the group
                    nxt = []
                    for jt, ps, c0, off in cur:
                        et = epool.tile([128, 2, QB], bf16, tag="et",
                                        name="et")
                        nc.scalar.activation(et[:, :, c0:QB], ps[:, :, c0:QB],
                                             Exp, scale=SCALE)
                        if off >= 0:
                            for s in (0, 1):
                                nc.vector.tensor_mul(
                                    et[:, s, c0:c0 + 128],
                                    et[:, s, c0:c0 + 128], tri[:])
                        nxt.append((jt, et, c0))
                    weave(WEAVE[b])
                    prev = nxt
                for jt, et, c0 in prev:
                    av_step(p, njt, avs, jt, et, c0)
                norms(avs, p, b)

            # ---- schedule: 16 sections, deps forced at section start,
            # deps prefetched TWO sections ahead + out-projections woven ----
            def deps_for(i):
                b, p = divmod(i, 4)
                us = [Unit(qk_unit(p, b)), Unit(qk_unit(p + 4, b))]
                if p == 0:
                    us += [Unit(v_unit(st)) for st in range(4 * b, 4 * b + 4)]
                return us

            # STARTUP (DMA-paced): the first two sections' deps are emitted
            # round-robin per k-tile so every weight/x arrival has a runnable
            # matmul -- a FIFO weave would head-of-line-block on the slowest
            # stream.  Their accumulators borrow the (idle) scores/av PSUM
            # slots so all 8 units are in flight at once; warm-up matmuls
            # (dep-free) front-fill the first ~4us.
            s_ps0 = sp.tile([128, 2, QB], f32, tag="ps", name="s_ps0")
            s_ps1 = sp.tile([128, 2, QB], f32, tag="ps", name="s_ps1")
            s_av0 = avp.tile([128, HPG, DH], f32, tag="av", name="s_av0")
            s_av1 = avp.tile([128, HPG, DH], f32, tag="av", name="s_av1")
            s_pp0 = pp.tile([128, HPG, DH], f32, tag="pp", name="s_pp0")
            s_pp1 = pp.tile([128, HPG, DH], f32, tag="pp", name="s_pp1")
            start_units = [
                Unit(qk_unit(0, 0, s_ps0[:, 0, :])),
                Unit(qk_unit(4, 0, s_ps0[:, 1, :])),
                Unit(v_unit(0, s_av0)), Unit(v_unit(1, s_av1)),
                Unit(v_unit(2, s_pp0)), Unit(v_unit(3, s_pp1)),
                Unit(qk_unit(1, 0, s_ps1[:, 0, :])),
                Unit(qk_unit(5, 0, s_ps1[:, 1, :])),
            ]
            warm = Unit(warm_unit())
            while not warm.done:
                warm.step()
            while not all(u.done for u in start_units):
                for u in start_units:
                    u.step()

            U = {0: [], 1: []}
            tail_boxes = []
            pending_outs = deque()
            nxt_dep = 2
            for i in range(16):
                b, p = divmod(i, 4)
                horizon = 3 if i <= 4 else 2
                while nxt_dep < 16 and nxt_dep <= i + horizon:
                    U[nxt_dep] = deps_for(nxt_dep)
                    fill_q.extend(U[nxt_dep])
                    nxt_dep += 1
                if i == 15:
                    # last block: ALL eight (st, n) out-projection chunks
                    # pre-run k<=2 during the final section (storing the
                    # partial straight to y so the two PSUM boxes recycle);
                    # only the k=3 tails remain after the final norms.
                    # Prepended so they outrank leftover block-2 fillers.
                    for st in (15, 14, 13, 12):
                        for n in (1, 0):
                            tail_boxes.append((st, n))
                            fill_q.appendleft(Unit(out_head(st, n)))
                    tail_boxes.reverse()
                # drip the previous block's out-projections (3 per section)
                # so their VectorE evictions don't burst at block boundaries
                for _ in range(3):
                    if pending_outs:
                        fill_q.append(Unit(out_unit(*pending_outs.popleft())))
                force(U[i])
                section(p, b)
                if p == 3 and b < 3:
                    pending_outs.extend(
                        (st, n) for st in range(4 * b, 4 * b + 4)
                        for n in range(2))
            for st, n in pending_outs:
                fill_q.append(Unit(out_unit(st, n)))
            # tail PSUM boxes: scores slots are idle after the final exps
            tailps = []
            for j in range(2):
                t = sp.tile([128, 2, QB], f32, tag="ps", name=f"tailps{j}")
                tailps += [t[:, 0, :], t[:, 1, :]]
            for j in range(2):
                t = pp.tile([128, 512], f32, tag="pp", name=f"tailpp{j}")
                tailps.append(t)
            for idx, (st, n) in enumerate(tail_boxes):
                fill_q.append(Unit(out_tail(st, n, tailps[idx % 6],
                                            idx % 2)))
            while fill_q:
                weave(1)

    nc.compile()
    return nc


def _shard_inputs(x, w_qkv, b_qkv, w_out):
    # keep key j (partition) <= query i (free column): upper triangle
    tri = np.triu(np.ones((128, 128))).astype(BF16)
    in_maps = []
    for c in range(N_CORES):
        b, g = c // G, c % G
        sl = slice(g * CG, (g + 1) * CG)
        bq = b_qkv[0 * D:1 * D][sl].reshape(CG // 128, 128).T
        bk = b_qkv[1 * D:2 * D][sl].reshape(CG // 128, 128).T
        in_maps.append({
            "xT": np.ascontiguousarray(x[b].T).astype(BF16),
            "wq": w_qkv[:, 0 * D:1 * D][:, sl].astype(BF16),
            "wk": w_qkv[:, 1 * D:2 * D][:, sl].astype(BF16),
            "wv": w_qkv[:, 2 * D:3 * D][:, sl].astype(BF16),
            "bqk": np.ascontiguousarray(
                np.concatenate([bq, bk], axis=1)).astype(np.float32),
            "wo": w_out[sl, :].astype(BF16),
            "tri": tri,
        })
    return in_maps


def kernel(x, w_qkv, b_qkv, w_out, b_out):
    from concourse.bass_utils import run_bass_kernel_spmd

    x = np.asarray(x, np.float32)
    w_qkv = np.asarray(w_qkv, np.float32)
    b_qkv = np.asarray(b_qkv, np.float32)
    w_out = np.asarray(w_out, np.float32)
    b_out = np.asarray(b_out, np.float32)

    if "nc" not in _cache:
        _cache["nc"] = _build_program()
    nc = _cache["nc"]

    in_maps = _shard_inputs(x, w_qkv, b_qkv, w_out)
    res = run_bass_kernel_spmd(nc, in_maps, core_ids=list(range(N_CORES)))
    _cache["last_result"] = res

    # v-projection bias contributes bv @ w_out (a constant row) to y:
    # attention weights sum to 1, so it survives softmax-averaging intact.
    bias = b_out + np.asarray(b_qkv[2 * D:3 * D], np.float32) @ w_out
    y = np.empty((B, S, D), np.float32)
    for b in range(B):
        r0, r1 = res.results[G * b], res.results[G * b + 1]
        y[b] = (r0["y"].astype(np.float32) + r1["y"].astype(np.float32)
                + bias)
        # k=3 tail of the last query block arrives in the separate y2 output
        y[b][S - QB:] += (r0["y2"].astype(np.float32)
                          + r1["y2"].astype(np.float32))
    return y


# revision 34
# speedup vs baseline: 1.0016x; 1.0016x over previous
"""Trainium2 Bass kernel for causal multi-head attention (software-pipelined).

Problem: B=4, S=2048, D=1024, H=16 heads (d_head=64), fp32 I/O.
    qkv = x @ w_qkv + b_qkv ; causal softmax attention ; out @ w_out + b_out

Sharding over 8 NeuronCores: data-parallel over batch (4) x tensor-parallel
over head-groups (2 groups of 8 heads). Core c handles batch c//2, head-group
c%2. No collectives: each core returns its partial out-projection
y_partial = attn_out_g @ w_out[rows_g]; the host sums the two group partials
per batch and adds b_out (plus b_v @ w_out -- see below).

Design notes (v2, evolved from the 311us phase-woven baseline):
  - 512-wide query blocks b=0..3; key j-tiles of 128; head pairs p=0..3
    (heads 2p, 2p+1 live in partition halves of qT/kT tiles; the two scores
    matmuls per j-tile are 64x128 row-tiles T0/T8 that stream CONCURRENTLY).
  - j-tiles processed in GROUPS OF TWO per PE mode: [scores jt0 + jt1]
    (64x128 tiling mode) then [AV jt0' + jt1' of the previous group]
    (128x128 mode).  Mode switches drain the PE array (~110ns each);
    grouping halves them vs per-j-tile alternation (trace: scores pair
    317ns vs 216 roofline = unhidden LDW + drain; AV s0 336 vs 216).
  - The two packed heads write ONE merged scores PSUM tile [128, 2, 512]
    (2 banks), one ScalarE exp covers both heads per j-tile.
  - AV accumulates [65, 512] per sub with a ones column in v producing the
    softmax denominator; max-free softmax (logits < ~7).
  - v-projection bias is folded OUT of the device: sum(attn)=1 makes
    av_norm(v_raw + bv) = av_norm(v_raw) + bv, and bv passes through the
    out-projection as the constant row bv @ w_out added host-side into
    b_out.  This deletes 16 K=1 N=512 bias matmuls (~5us PE).
  - Projections (qk via stationary w, v natural) and the out-projection are
    split into small units woven between attention groups from a generator
    queue; deps are prefetched TWO sections ahead so block transitions
    (2 qk + 4 v units = ~13us of PE) never starve the weave.
  - qk bias+PSUM-drain runs on VectorE (tensor_scalar_add), ScalarE stays
    exp-only.
  - PSUM budget: scores 2x2 + av 2x1 + proj/out 2x1 = 8 banks exactly.
  - DMA: inputs split across both HWDGE queues (Sync: wq + x[:, :512] the
    lead needs, then x tails + wo; Scalar: small consts + wk + wv).
    bq/bk packed host-side into one [128, 8] tensor.  A dummy exp preloads
    the ACT table during the DMA lead; a dependency-free 36-matmul warm-up
    block on a MEMSET tile (no DMA gate) un-throttles the HAM clock gate
    ~7us earlier than the tri-gated variant.
  - Output y is stored bf16 (halves the 8MB/core store drain; host sums
    the two partials in f32; adds ~0.1% rounding, budget is 2e-2).
  - Tail: the last block's first two out-projection chunks split into a
    k<=2 partial woven during the final section + a k=3 tail after the
    final norms (accumulation-group semaphore waits hoist to the group
    head and would otherwise serialize behind the final normalization).
"""

import sys

if "/opt/trn_rl_repo" not in sys.path:
    sys.path.insert(0, "/opt/trn_rl_repo")

from collections import deque

import numpy as np
import ml_dtypes

B, S, D = 4, 2048, 1024
H, DH = 16, 64
G = 2                # tensor-parallel head groups
HPG = H // G         # heads per group (8)
CG = HPG * DH        # channel cols per group (512)
N_CORES = 8
BF16 = ml_dtypes.bfloat16

KT = D // 128        # 8 contraction k-tiles for the projections
QB = 512             # query block width
NB = S // QB         # 4 query blocks

_cache = {}


def _build_program():
    import concourse.tile as tile
    from concourse import bacc, mybir

    f32 = mybir.dt.float32
    bf16 = mybir.dt.bfloat16
    Exp = mybir.ActivationFunctionType.Exp
    Copy = mybir.ActivationFunctionType.Copy
    SCALE = float(DH) ** -0.5

    nc = bacc.Bacc("TRN2", target_bir_lowering=False, debug=False,
                   num_devices=N_CORES)

    xT_d = nc.dram_tensor("xT", [D, S], bf16, kind="ExternalInput").ap()
    wq_d = nc.dram_tensor("wq", [D, CG], bf16, kind="ExternalInput").ap()
    wk_d = nc.dram_tensor("wk", [D, CG], bf16, kind="ExternalInput").ap()
    wv_d = nc.dram_tensor("wv", [D, CG], bf16, kind="ExternalInput").ap()
    # bq/bk packed as one [128, 8] tensor (col m = bq chunk m, col 4+m = bk
    # chunk m): a single DMA instead of eight 4-byte-element ones
    bqk_d = nc.dram_tensor("bqk", [128, 8], f32, kind="ExternalInput").ap()
    wo_d = nc.dram_tensor("wo", [CG, D], bf16, kind="ExternalInput").ap()
    tri_d = nc.dram_tensor("tri", [128, 128], bf16, kind="ExternalInput").ap()
    y_d = nc.dram_tensor("y", [S, D], bf16, kind="ExternalOutput").ap()
    # k=3 tail contributions of the last query block (host adds to y rows
    # 1536:2048 -- lets the post-final-norms eviction run on the idle
    # ScalarE instead of the busy VectorE)
    y2_d = nc.dram_tensor("y2", [QB, D], bf16, kind="ExternalOutput").ap()

    with tile.TileContext(nc) as tc:
        with (
            tc.tile_pool(name="consts", bufs=1) as cpool,
            tc.tile_pool(name="acts", bufs=1) as apool,
            tc.tile_pool(name="exps", bufs=6) as epool,
            tc.tile_pool(name="small", bufs=2) as spool,
            tc.tile_pool(name="rbc", bufs=2) as rpool,
            tc.tile_pool(name="ystage", bufs=3) as ypool,
            tc.tile_pool(name="psum_s", bufs=2, space="PSUM") as sp,
            tc.tile_pool(name="psum_av", bufs=2, space="PSUM") as avp,
            tc.tile_pool(name="psum_p", bufs=2, space="PSUM") as pp,
        ):
            # ---- DMA loads, split across the two HWDGE queues (Sync +
            # Scalar) to double input bandwidth. ----
            ones_row = cpool.tile([1, 128], bf16, tag="ones")
            nc.gpsimd.memset(ones_row[:], 1.0)
            # warm-up stationary: memset (NOT DMA-gated) so the HAM warm-up
            # can start during the bass preamble
            wrm = cpool.tile([128, 128], bf16, tag="wrm")
            nc.gpsimd.memset(wrm[:], 0.25)
            # dummy exp on a const tile: forces the ACT table load (~2.7us)
            # to happen during the DMA lead, before the ScalarE-queue DMAs
            warm_exp = spool.tile([1, 128], f32, tag="wexp", name="warm_exp")
            nc.scalar.activation(warm_exp[:], ones_row[:], Exp, scale=1.0)

            bqk = cpool.tile([128, 8], f32, tag="bqk")
            nc.scalar.dma_start(bqk[:], bqk_d[:])
            bqc = [bqk[:, m:m + 1] for m in range(4)]
            bkc = [bqk[:, 4 + m:5 + m] for m in range(4)]
            tri = cpool.tile([128, 128], bf16, tag="tri")
            nc.scalar.dma_start(tri[:], tri_d[:])

            wq, wk, wv, xt = [], [], [], []
            for k in range(KT):
                rows = slice(k * 128, (k + 1) * 128)
                t = cpool.tile([128, CG], bf16, tag=f"wq{k}", name=f"wq{k}")
                nc.sync.dma_start(t[:], wq_d[rows, :])
                wq.append(t)
                t = cpool.tile([128, CG], bf16, tag=f"wk{k}", name=f"wk{k}")
                nc.scalar.dma_start(t[:], wk_d[rows, :])
                wk.append(t)
                t = cpool.tile([128, CG], bf16, tag=f"wv{k}", name=f"wv{k}")
                nc.scalar.dma_start(t[:], wv_d[rows, :])
                wv.append(t)
                t = cpool.tile([128, S], bf16, tag=f"xt{k}", name=f"xt{k}")
                nc.sync.dma_start(t[:, 0:QB], xT_d[rows, 0:QB])
                xt.append(t)
            for k in range(KT):
                nc.sync.dma_start(xt[k][:, QB:S],
                                  xT_d[k * 128:(k + 1) * 128, QB:S])
            wo = []
            for k in range(CG // 128):
                t = cpool.tile([128, D], bf16, tag=f"wo{k}", name=f"wo{k}")
                nc.sync.dma_start(t[:], wo_d[k * 128:(k + 1) * 128, :])
                wo.append(t)

            # ---- persistent activations ----
            qT = [apool.tile([128, S], bf16, tag=f"qT{m}", name=f"qT{m}")
                  for m in range(CG // 128)]
            kTt = [apool.tile([128, S], bf16, tag=f"kT{m}", name=f"kT{m}")
                   for m in range(CG // 128)]
            # v with a ones column per head: [s, h, 0:64] = v_h, [s, h, 64] = 1
            vst = [apool.tile([128, HPG, DH + 1], bf16, tag=f"v{st}",
                              name=f"v{st}")
                   for st in range(S // 128)]
            aoT = [apool.tile([128, S], bf16, tag=f"aoT{m}", name=f"aoT{m}")
                   for m in range(CG // 128)]

            for st in range(S // 128):
                nc.gpsimd.memset(vst[st][:, :, DH:DH + 1], 1.0)

            # ---- filler units (generators yielding between PE matmuls) ----
            def qk_unit(m, n, acc=None):
                wt, bc, dst = (wq, bqc, qT) if m < 4 else (wk, bkc, kTt)
                mi = m % 4
                if acc is None:
                    acc = pp.tile([128, QB], f32, tag="pp", name=f"qk{m}_{n}")
                for k in range(KT):
                    nc.tensor.matmul(
                        acc[:], wt[k][:, mi * 128:(mi + 1) * 128],
                        xt[k][:, n * QB:(n + 1) * QB],
                        start=(k == 0), stop=(k == KT - 1))
                    yield
                nc.vector.tensor_scalar_add(
                    dst[mi][:, n * QB:(n + 1) * QB], acc[:], bc[mi])
                yield

            def v_unit(st, acc=None):
                if acc is None:
                    acc = pp.tile([128, HPG, DH], f32, tag="pp",
                                  name=f"vacc{st}")
                for k in range(KT):
                    nc.tensor.matmul(
                        acc[:, :, :], xt[k][:, st * 128:(st + 1) * 128],
                        wv[k][:], start=(k == 0), stop=(k == KT - 1))
                    yield
                nc.vector.tensor_copy(vst[st][:, :, 0:DH], acc[:, :, :])
                yield

            def out_unit(st, n):
                yp = pp.tile([128, 512], f32, tag="pp", name=f"yp{st}_{n}")
                for k in range(CG // 128):
                    nc.tensor.matmul(
                        yp[:], aoT[k][:, st * 128:(st + 1) * 128],
                        wo[k][:, n * 512:(n + 1) * 512],
                        start=(k == 0), stop=(k == CG // 128 - 1))
                    yield
                ys = ypool.tile([128, 512], bf16, tag="ys", name=f"ys{st}_{n}")
                nc.vector.tensor_copy(ys[:], yp[:])
                nc.sync.dma_start(
                    y_d[st * 128:(st + 1) * 128, n * 512:(n + 1) * 512], ys[:])
                yield

            # split variants for the LAST block: the k=0..2 partial (deps:
            # head pairs 0..2 only) weaves during the final section and is
            # EVICTED to SBUF bf16, freeing its PSUM box so all 8 chunks can
            # pre-run; the k=3 matmul + add + store runs after the final
            # norm.  Costs one extra bf16 rounding on 3/4 of y (~0.1%).
            def out_head(st, n):
                yp = pp.tile([128, 512], f32, tag="pp", name=f"yph{st}_{n}")
                for k in range(3):
                    nc.tensor.matmul(
                        yp[:], aoT[k][:, st * 128:(st + 1) * 128],
                        wo[k][:, n * 512:(n + 1) * 512],
                        start=(k == 0), stop=(k == 2))
                    yield
                part = ypool.tile([128, 512], bf16, tag="ys",
                                  name=f"part{st}_{n}")
                nc.vector.tensor_copy(part[:], yp[:])
                nc.sync.dma_start(
                    y_d[st * 128:(st + 1) * 128, n * 512:(n + 1) * 512],
                    part[:])
                yield

            def out_tail(st, n, yp, eng):
                # yp: a dedicated PSUM view (scores banks are free after the
                # final exps) so all 8 tail matmuls issue back-to-back; the
                # evictions alternate between the (exp-done, idle) ScalarE
                # and VectorE into y2, which the host adds to y rows
                # 1536:2048.
                nc.tensor.matmul(
                    yp[:], aoT[3][:, st * 128:(st + 1) * 128],
                    wo[3][:, n * 512:(n + 1) * 512],
                    start=True, stop=True)
                yield
                ys = ypool.tile([128, 512], bf16, tag="ys2", bufs=8,
                                name=f"ys2_{st}_{n}")
                if eng == 0:
                    nc.scalar.activation(ys[:], yp[:], Copy, scale=1.0)
                else:
                    nc.vector.tensor_copy(ys[:], yp[:])
                nc.sync.dma_start(
                    y2_d[(st - 12) * 128:(st - 11) * 128,
                         n * 512:(n + 1) * 512], ys[:])
                yield

            def warm_unit():
                # ~4us of dependency-free full-array matmuls: fills the
                # preamble/DMA lead and un-throttles the HAM clock gate
                # (which watches PE array activity) before dense work.
                wp = pp.tile([128, 128], f32, tag="pp", name="warmps")
                for i in range(36):
                    nc.tensor.matmul(wp[:], wrm[:], wrm[:],
                                     start=True, stop=True)
                    yield

            class Unit:
                __slots__ = ("gen", "done")

                def __init__(self, gen):
                    self.gen = gen
                    self.done = False

                def step(self):
                    if self.done:
                        return False
                    try:
                        next(self.gen)
                        return True
                    except StopIteration:
                        self.done = True
                        return False

            fill_q = deque()

            def weave(n):
                while n > 0 and fill_q:
                    u = fill_q[0]
                    if u.step():
                        n -= 1
                    else:
                        fill_q.popleft()

            def force(units):
                while not all(u.done for u in units):
                    weave(1)
                    if not fill_q:
                        break

            # ---- attention section for (head pair p, query block b) ----
            def norms(avs, p, b):
                # normalize by the broadcast fast-reciprocal of the ones-row
                # denominator; custom-DVE recip must not read PSUM: stage the
                # row in SBUF first.  The two subs' chains are interleaved so
                # the gpsimd broadcasts overlap the DVE ops.
                final = (p == 3 and b == 3)
                dn, rc, rb = [None, None], [None, None], [None, None]
                for s in (0, 1):
                    dn[s] = spool.tile([1, QB], f32, tag=f"dn{s}", name="dn")
                    rc[s] = spool.tile([1, QB], f32, tag=f"rc{s}", name="rc")
                if final:
                    # exps are done; ScalarE is idle -- stage the two
                    # denominators in parallel across engines (s=1's whole
                    # chain runs first on VectorE so it never waits s=0's
                    # ScalarE copy)
                    nc.scalar.activation(dn[0][:], avs[0][DH:DH + 1, :],
                                         Copy, scale=1.0)
                    nc.vector.tensor_copy(dn[1][:], avs[1][DH:DH + 1, :])
                    nc.vector.reciprocal_approx_fast(rc[1][:], dn[1][:])
                    nc.vector.reciprocal_approx_fast(rc[0][:], dn[0][:])
                else:
                    for s in (0, 1):
                        nc.vector.tensor_copy(dn[s][:], avs[s][DH:DH + 1, :])
                        nc.vector.reciprocal_approx_fast(rc[s][:], dn[s][:])
                for s in (0, 1):
                    rb[s] = rpool.tile([DH, QB], f32, tag=f"rb{s}", name="rb")
                    nc.gpsimd.partition_broadcast(rb[s][:], rc[s][:])
                if not (p == 3 and b == 3):
                    for s in (0, 1):
                        nc.vector.tensor_mul(
                            aoT[p][DH * s:DH * s + DH, b * QB:(b + 1) * QB],
                            avs[s][0:DH, :], rb[s][:])
                else:
                    # final section: per 128-query chunk, st-ascending, so
                    # the out-projection k=3 tails unblock progressively
                    for c in range(4):
                        cs = slice(c * 128, (c + 1) * 128)
                        for s in (0, 1):
                            nc.vector.tensor_mul(
                                aoT[p][DH * s:DH * s + DH,
                                       b * QB + c * 128:
                                       b * QB + (c + 1) * 128],
                                avs[s][0:DH, cs], rb[s][:, cs])

            def av_step(p, njt, avs, jt, et, c0):
                for s in (0, 1):
                    nc.tensor.matmul(
                        avs[s][:, c0:QB], vst[jt][:, 2 * p + s, :],
                        et[:, s, c0:QB],
                        start=(jt == 0), stop=(jt == njt - 1))

            WEAVE = (8, 6, 4, 4)   # filler steps per group, by block

            def section(p, b):
                njt = 4 * (b + 1)
                avs = [avp.tile([DH + 1, QB], f32, tag="av",
                                name=f"av{p}_{b}_{s}") for s in (0, 1)]
                prev = []
                for g in range(njt // 2):
                    # scores for both j-tiles of the group (64x128 tiling
                    # mode, pairs stream concurrently; 2nd pair's LDW hides
                    # behind the 1st pair's matmuls)
                    cur = []
                    for jt in (2 * g, 2 * g + 1):
                        off = jt - 4 * b
                        c0 = 128 * off if off > 0 else 0
                        ps = sp.tile([128, 2, QB], f32, tag="ps",
                                     name=f"ps{p}_{b}_{jt}")
                        for s in (0, 1):
                            nc.tensor.matmul(
                                ps[:, s, c0:QB],
                                kTt[p][DH * s:DH * s + DH,
                                       jt * 128:(jt + 1) * 128],
                                qT[p][DH * s:DH * s + DH,
                                      b * QB + c0:(b + 1) * QB],
                                start=True, stop=True)
                        cur.append((jt, ps, c0, off))
                    # previous group's AV pairs (128x128 mode)
                    for jt, et, c0 in prev:
                        av_step(p, njt, avs, jt, et, c0)
                    # exps (ScalarE) + diagonal masks (VectorE) for the group
                    nxt = []
                    for jt, ps, c0, off in cur:
                        et = epool.tile([128, 2, QB], bf16, tag="et",
                                        name="et")
                        nc.scalar.activation(et[:, :, c0:QB], ps[:, :, c0:QB],
                                             Exp, scale=SCALE)
                        if off >= 0:
                            for s in (0, 1):
                                nc.vector.tensor_mul(
                                    et[:, s, c0:c0 + 128],
                                    et[:, s, c0:c0 + 128], tri[:])
                        nxt.append((jt, et, c0))
                    weave(WEAVE[b])
                    prev = nxt
                for jt, et, c0 in prev:
                    av_step(p, njt, avs, jt, et, c0)
                norms(avs, p, b)

            # ---- schedule: 16 sections, deps forced at section start,
            # deps prefetched TWO sections ahead + out-projections woven ----
            def deps_for(i):
                b, p = divmod(i, 4)
                us = [Unit(qk_unit(p, b)), Unit(qk_unit(p + 4, b))]
                if p == 0:
                    us += [Unit(v_unit(st)) for st in range(4 * b, 4 * b + 4)]
                return us

            # STARTUP (DMA-paced): the first two sections' deps are emitted
            # round-robin per k-tile so every weight/x arrival has a runnable
            # matmul -- a FIFO weave would head-of-line-block on the slowest
            # stream.  Their accumulators borrow the (idle) scores/av PSUM
            # slots so all 8 units are in flight at once; warm-up matmuls
            # (dep-free) front-fill the first ~4us.
            s_ps0 = sp.tile([128, 2, QB], f32, tag="ps", name="s_ps0")
            s_ps1 = sp.tile([128, 2, QB], f32, tag="ps", name="s_ps1")
            s_av0 = avp.tile([128, HPG, DH], f32, tag="av", name="s_av0")
            s_av1 = avp.tile([128, HPG, DH], f32, tag="av", name="s_av1")
            s_pp0 = pp.tile([128, HPG, DH], f32, tag="pp", name="s_pp0")
            s_pp1 = pp.tile([128, HPG, DH], f32, tag="pp", name="s_pp1")
            start_units = [
                Unit(qk_unit(0, 0, s_ps0[:, 0, :])),
                Unit(qk_unit(4, 0, s_ps0[:, 1, :])),
                Unit(v_unit(0, s_av0)), Unit(v_unit(1, s_av1)),
                Unit(v_unit(2, s_pp0)), Unit(v_unit(3, s_pp1)),
                Unit(qk_unit(1, 0, s_ps1[:, 0, :])),
                Unit(qk_unit(5, 0, s_ps1[:, 1, :])),
            ]
            warm = Unit(warm_unit())
            while not warm.done:
                warm.step()
            while not all(u.done for u in start_units):
                for u in start_units:
                    u.step()

            U = {0: [], 1: []}
            tail_boxes = []
            pending_outs = deque()
            nxt_dep = 2
            for i in range(16):
                b, p = divmod(i, 4)
                horizon = 3 if i <= 4 else 2
                while nxt_dep < 16 and nxt_dep <= i + horizon:
                    U[nxt_dep] = deps_for(nxt_dep)
                    fill_q.extend(U[nxt_dep])
                    nxt_dep += 1
                if i == 15:
                    # last block: ALL eight (st, n) out-projection chunks
                    # pre-run k<=2 during the final section (storing the
                    # partial straight to y so the two PSUM boxes recycle);
                    # only the k=3 tails remain after the final norms.
                    # Prepended so they outrank leftover block-2 fillers.
                    for st in (15, 14, 13, 12):
                        for n in (1, 0):
                            tail_boxes.append((st, n))
                            fill_q.appendleft(Unit(out_head(st, n)))
                    tail_boxes.reverse()
                # drip the previous block's out-projections (3 per section)
                # so their VectorE evictions don't burst at block boundaries
                for _ in range(3):
                    if pending_outs:
                        fill_q.append(Unit(out_unit(*pending_outs.popleft())))
                force(U[i])
                section(p, b)
                if p == 3 and b < 3:
                    pending_outs.extend(
                        (st, n) for st in range(4 * b, 4 * b + 4)
                        for n in range(2))
            for st, n in pending_outs:
                fill_q.append(Unit(out_unit(st, n)))
            # tail PSUM boxes: scores slots are idle after the final exps
            tailps = []
            for j in range(2):
                t = sp.tile([128, 2, QB], f32, tag="ps", name=f"tailps{j}")
                tailps += [t[:, 0, :], t[:, 1, :]]
            for j in range(2):
                t = pp.tile([128, 512], f32, tag="pp", name=f"tailpp{j}")
                tailps.append(t)
            for idx, (st, n) in enumerate(tail_boxes):
                fill_q.append(Unit(out_tail(st, n, tailps[idx % 6],
                                            idx % 2)))
            while fill_q:
                weave(1)

    nc.compile()
    return nc


def _shard_inputs(x, w_qkv, b_qkv, w_out):
    # keep key j (partition) <= query i (free column): upper triangle
    tri = np.triu(np.ones((128, 128))).astype(BF16)
    in_maps = []
    for c in range(N_CORES):
        b, g = c // G, c % G
        sl = slice(g * CG, (g + 1) * CG)
        bq = b_qkv[0 * D:1 * D][sl].reshape(CG // 128, 128).T
        bk = b_qkv[1 * D:2 * D][sl].reshape(CG // 128, 128).T
        in_maps.append({
            "xT": np.ascontiguousarray(x[b].T).astype(BF16),
            "wq": w_qkv[:, 0 * D:1 * D][:, sl].astype(BF16),
            "wk": w_qkv[:, 1 * D:2 * D][:, sl].astype(BF16),
            "wv": w_qkv[:, 2 * D:3 * D][:, sl].astype(BF16),
            "bqk": np.ascontiguousarray(
                np.concatenate([bq, bk], axis=1)).astype(np.float32),
            "wo": w_out[sl, :].astype(BF16),
            "tri": tri,
        })
    return in_maps


def kernel(x, w_qkv, b_qkv, w_out, b_out):
    from concourse.bass_utils import run_bass_kernel_spmd

    x = np.asarray(x, np.float32)
    w_qkv = np.asarray(w_qkv, np.float32)
    b_qkv = np.asarray(b_qkv, np.float32)
    w_out = np.asarray(w_out, np.float32)
    b_out = np.asarray(b_out, np.float32)

    if "nc" not in _cache:
        _cache["nc"] = _build_program()
    nc = _cache["nc"]

    in_maps = _shard_inputs(x, w_qkv, b_qkv, w_out)
    res = run_bass_kernel_spmd(nc, in_maps, core_ids=list(range(N_CORES)))
    _cache["last_result"] = res

    # v-projection bias contributes bv @ w_out (a constant row) to y:
    # attention weights sum to 1, so it survives softmax-averaging intact.
    bias = b_out + np.asarray(b_qkv[2 * D:3 * D], np.float32) @ w_out
    y = np.empty((B, S, D), np.float32)
    for b in range(B):
        r0, r1 = res.results[G * b], res.results[G * b + 1]
        y[b] = (r0["y"].astype(np.float32) + r1["y"].astype(np.float32)
                + bias)
        # k=3 tail of the last query block arrives in the separate y2 output
        y[b][S - QB:] += (r0["y2"].astype(np.float32)
                          + r1["y2"].astype(np.float32))
    return y


# revision 35
# speedup vs baseline: 1.0174x; 1.0158x over previous
"""Trainium2 Bass kernel for causal multi-head attention (software-pipelined).

Problem: B=4, S=2048, D=1024, H=16 heads (d_head=64), fp32 I/O.
    qkv = x @ w_qkv + b_qkv ; causal softmax attention ; out @ w_out + b_out

Sharding over 8 NeuronCores: data-parallel over batch (4) x tensor-parallel
over head-groups (2 groups of 8 heads). Core c handles batch c//2, head-group
c%2. No collectives: each core returns its partial out-projection
y_partial = attn_out_g @ w_out[rows_g]; the host sums the two group partials
per batch and adds b_out (plus b_v @ w_out -- see below).

Design notes (v2, evolved from the 311us phase-woven baseline):
  - 512-wide query blocks b=0..3; key j-tiles of 128; head pairs p=0..3
    (heads 2p, 2p+1 live in partition halves of qT/kT tiles; the two scores
    matmuls per j-tile are 64x128 row-tiles T0/T8 that stream CONCURRENTLY).
  - j-tiles processed in GROUPS OF TWO per PE mode: [scores jt0 + jt1]
    (64x128 tiling mode) then [AV jt0' + jt1' of the previous group]
    (128x128 mode).  Mode switches drain the PE array (~110ns each);
    grouping halves them vs per-j-tile alternation (trace: scores pair
    317ns vs 216 roofline = unhidden LDW + drain; AV s0 336 vs 216).
  - The two packed heads write ONE merged scores PSUM tile [128, 2, 512]
    (2 banks), one ScalarE exp covers both heads per j-tile.
  - AV accumulates [65, 512] per sub with a ones column in v producing the
    softmax denominator; max-free softmax (logits < ~7).
  - v-projection bias is folded OUT of the device: sum(attn)=1 makes
    av_norm(v_raw + bv) = av_norm(v_raw) + bv, and bv passes through the
    out-projection as the constant row bv @ w_out added host-side into
    b_out.  This deletes 16 K=1 N=512 bias matmuls (~5us PE).
  - Projections (qk via stationary w, v natural) and the out-projection are
    split into small units woven between attention groups from a generator
    queue; deps are prefetched TWO sections ahead so block transitions
    (2 qk + 4 v units = ~13us of PE) never starve the weave.
  - qk bias+PSUM-drain runs on VectorE (tensor_scalar_add), ScalarE stays
    exp-only.
  - PSUM budget: scores 2x2 + av 2x1 + proj/out 2x1 = 8 banks exactly.
  - DMA: inputs split across both HWDGE queues (Sync: wq + x[:, :512] the
    lead needs, then x tails + wo; Scalar: small consts + wk + wv).
    bq/bk packed host-side into one [128, 8] tensor.  A dummy exp preloads
    the ACT table during the DMA lead; a dependency-free 36-matmul warm-up
    block on a MEMSET tile (no DMA gate) un-throttles the HAM clock gate
    ~7us earlier than the tri-gated variant.
  - Output y is stored bf16 (halves the 8MB/core store drain; host sums
    the two partials in f32; adds ~0.1% rounding, budget is 2e-2).
  - Tail: the last block's first two out-projection chunks split into a
    k<=2 partial woven during the final section + a k=3 tail after the
    final norms (accumulation-group semaphore waits hoist to the group
    head and would otherwise serialize behind the final normalization).
"""

import sys

if "/opt/trn_rl_repo" not in sys.path:
    sys.path.insert(0, "/opt/trn_rl_repo")

from collections import deque

import numpy as np
import ml_dtypes

B, S, D = 4, 2048, 1024
H, DH = 16, 64
G = 2                # tensor-parallel head groups
HPG = H // G         # heads per group (8)
CG = HPG * DH        # channel cols per group (512)
N_CORES = 8
BF16 = ml_dtypes.bfloat16

KT = D // 128        # 8 contraction k-tiles for the projections
QB = 512             # query block width
NB = S // QB         # 4 query blocks

_cache = {}


def _build_program():
    import concourse.tile as tile
    from concourse import bacc, mybir

    f32 = mybir.dt.float32
    bf16 = mybir.dt.bfloat16
    Exp = mybir.ActivationFunctionType.Exp
    SCALE = float(DH) ** -0.5

    nc = bacc.Bacc("TRN2", target_bir_lowering=False, debug=False,
                   num_devices=N_CORES)

    xT_d = nc.dram_tensor("xT", [D, S], bf16, kind="ExternalInput").ap()
    wq_d = nc.dram_tensor("wq", [D, CG], bf16, kind="ExternalInput").ap()
    wk_d = nc.dram_tensor("wk", [D, CG], bf16, kind="ExternalInput").ap()
    wv_d = nc.dram_tensor("wv", [D, CG], bf16, kind="ExternalInput").ap()
    # bq/bk packed as one [128, 8] tensor (col m = bq chunk m, col 4+m = bk
    # chunk m): a single DMA instead of eight 4-byte-element ones
    bqk_d = nc.dram_tensor("bqk", [128, 8], f32, kind="ExternalInput").ap()
    wo_d = nc.dram_tensor("wo", [CG, D], bf16, kind="ExternalInput").ap()
    tri_d = nc.dram_tensor("tri", [128, 128], bf16, kind="ExternalInput").ap()
    y_d = nc.dram_tensor("y", [S, D], bf16, kind="ExternalOutput").ap()

    with tile.TileContext(nc) as tc:
        with (
            tc.tile_pool(name="consts", bufs=1) as cpool,
            tc.tile_pool(name="acts", bufs=1) as apool,
            tc.tile_pool(name="exps", bufs=6) as epool,
            tc.tile_pool(name="small", bufs=2) as spool,
            tc.tile_pool(name="rbc", bufs=2) as rpool,
            tc.tile_pool(name="ystage", bufs=3) as ypool,
            tc.tile_pool(name="psum_s", bufs=2, space="PSUM") as sp,
            tc.tile_pool(name="psum_av", bufs=2, space="PSUM") as avp,
            tc.tile_pool(name="psum_p", bufs=2, space="PSUM") as pp,
        ):
            # ---- DMA loads, split across the two HWDGE queues (Sync +
            # Scalar) to double input bandwidth. ----
            ones_row = cpool.tile([1, 128], bf16, tag="ones")
            nc.gpsimd.memset(ones_row[:], 1.0)
            # warm-up stationary: memset (NOT DMA-gated) so the HAM warm-up
            # can start during the bass preamble
            wrm = cpool.tile([128, 128], bf16, tag="wrm")
            nc.gpsimd.memset(wrm[:], 0.25)
            # dummy exp on a const tile: forces the ACT table load (~2.7us)
            # to happen during the DMA lead, before the ScalarE-queue DMAs
            warm_exp = spool.tile([1, 128], f32, tag="wexp", name="warm_exp")
            nc.scalar.activation(warm_exp[:], ones_row[:], Exp, scale=1.0)

            bqk = cpool.tile([128, 8], f32, tag="bqk")
            nc.scalar.dma_start(bqk[:], bqk_d[:])
            bqc = [bqk[:, m:m + 1] for m in range(4)]
            bkc = [bqk[:, 4 + m:5 + m] for m in range(4)]
            tri = cpool.tile([128, 128], bf16, tag="tri")
            nc.scalar.dma_start(tri[:], tri_d[:])

            wq, wk, wv, xt = [], [], [], []
            for k in range(KT):
                rows = slice(k * 128, (k + 1) * 128)
                t = cpool.tile([128, CG], bf16, tag=f"wq{k}", name=f"wq{k}")
                nc.sync.dma_start(t[:], wq_d[rows, :])
                wq.append(t)
                t = cpool.tile([128, CG], bf16, tag=f"wk{k}", name=f"wk{k}")
                nc.scalar.dma_start(t[:], wk_d[rows, :])
                wk.append(t)
                t = cpool.tile([128, CG], bf16, tag=f"wv{k}", name=f"wv{k}")
                nc.scalar.dma_start(t[:], wv_d[rows, :])
                wv.append(t)
                t = cpool.tile([128, S], bf16, tag=f"xt{k}", name=f"xt{k}")
                nc.sync.dma_start(t[:, 0:QB], xT_d[rows, 0:QB])
                xt.append(t)
            for k in range(KT):
                nc.sync.dma_start(xt[k][:, QB:S],
                                  xT_d[k * 128:(k + 1) * 128, QB:S])
            wo = []
            for k in range(CG // 128):
                t = cpool.tile([128, D], bf16, tag=f"wo{k}", name=f"wo{k}")
                nc.sync.dma_start(t[:], wo_d[k * 128:(k + 1) * 128, :])
                wo.append(t)

            # ---- persistent activations ----
            qT = [apool.tile([128, S], bf16, tag=f"qT{m}", name=f"qT{m}")
                  for m in range(CG // 128)]
            kTt = [apool.tile([128, S], bf16, tag=f"kT{m}", name=f"kT{m}")
                   for m in range(CG // 128)]
            # v with a ones column per head: [s, h, 0:64] = v_h, [s, h, 64] = 1
            vst = [apool.tile([128, HPG, DH + 1], bf16, tag=f"v{st}",
                              name=f"v{st}")
                   for st in range(S // 128)]
            aoT = [apool.tile([128, S], bf16, tag=f"aoT{m}", name=f"aoT{m}")
                   for m in range(CG // 128)]

            for st in range(S // 128):
                nc.gpsimd.memset(vst[st][:, :, DH:DH + 1], 1.0)

            # ---- filler units (generators yielding between PE matmuls) ----
            def qk_unit(m, n, acc=None):
                wt, bc, dst = (wq, bqc, qT) if m < 4 else (wk, bkc, kTt)
                mi = m % 4
                if acc is None:
                    acc = pp.tile([128, QB], f32, tag="pp", name=f"qk{m}_{n}")
                for k in range(KT):
                    nc.tensor.matmul(
                        acc[:], wt[k][:, mi * 128:(mi + 1) * 128],
                        xt[k][:, n * QB:(n + 1) * QB],
                        start=(k == 0), stop=(k == KT - 1))
                    yield
                nc.vector.tensor_scalar_add(
                    dst[mi][:, n * QB:(n + 1) * QB], acc[:], bc[mi])
                yield

            def v_unit(st, acc=None):
                if acc is None:
                    acc = pp.tile([128, HPG, DH], f32, tag="pp",
                                  name=f"vacc{st}")
                for k in range(KT):
                    nc.tensor.matmul(
                        acc[:, :, :], xt[k][:, st * 128:(st + 1) * 128],
                        wv[k][:], start=(k == 0), stop=(k == KT - 1))
                    yield
                nc.vector.tensor_copy(vst[st][:, :, 0:DH], acc[:, :, :])
                yield

            def out_unit(st, n):
                yp = pp.tile([128, 512], f32, tag="pp", name=f"yp{st}_{n}")
                for k in range(CG // 128):
                    nc.tensor.matmul(
                        yp[:], aoT[k][:, st * 128:(st + 1) * 128],
                        wo[k][:, n * 512:(n + 1) * 512],
                        start=(k == 0), stop=(k == CG // 128 - 1))
                    yield
                ys = ypool.tile([128, 512], bf16, tag="ys", name=f"ys{st}_{n}")
                nc.vector.tensor_copy(ys[:], yp[:])
                nc.sync.dma_start(
                    y_d[st * 128:(st + 1) * 128, n * 512:(n + 1) * 512], ys[:])
                yield

            # split variants for the LAST block: the k=0..2 partial (deps:
            # head pairs 0..2 only) weaves during the final section and is
            # EVICTED to SBUF bf16, freeing its PSUM box so all 8 chunks can
            # pre-run; the k=3 matmul + add + store runs after the final
            # norm.  Costs one extra bf16 rounding on 3/4 of y (~0.1%).
            def out_head(st, n, box):
                yp = pp.tile([128, 512], f32, tag="pp", name=f"yph{st}_{n}")
                for k in range(3):
                    nc.tensor.matmul(
                        yp[:], aoT[k][:, st * 128:(st + 1) * 128],
                        wo[k][:, n * 512:(n + 1) * 512],
                        start=(k == 0), stop=(k == 2))
                    yield
                part = ypool.tile([128, 512], bf16, tag=f"part{st}_{n}",
                                  bufs=1, name=f"part{st}_{n}")
                box.append(part)
                nc.vector.tensor_copy(part[:], yp[:])
                yield

            def out_tail(st, n, box):
                yp = pp.tile([128, 512], f32, tag="pp", name=f"ypt{st}_{n}")
                nc.tensor.matmul(
                    yp[:], aoT[3][:, st * 128:(st + 1) * 128],
                    wo[3][:, n * 512:(n + 1) * 512],
                    start=True, stop=True)
                yield
                ys = ypool.tile([128, 512], bf16, tag="ys", name=f"ys{st}_{n}")
                nc.vector.tensor_add(ys[:], yp[:], box[0][:])
                nc.sync.dma_start(
                    y_d[st * 128:(st + 1) * 128, n * 512:(n + 1) * 512], ys[:])
                yield

            def warm_unit():
                # ~4us of dependency-free full-array matmuls: fills the
                # preamble/DMA lead and un-throttles the HAM clock gate
                # (which watches PE array activity) before dense work.
                wp = pp.tile([128, 128], f32, tag="pp", name="warmps")
                for i in range(36):
                    nc.tensor.matmul(wp[:], wrm[:], wrm[:],
                                     start=True, stop=True)
                    yield

            class Unit:
                __slots__ = ("gen", "done")

                def __init__(self, gen):
                    self.gen = gen
                    self.done = False

                def step(self):
                    if self.done:
                        return False
                    try:
                        next(self.gen)
                        return True
                    except StopIteration:
                        self.done = True
                        return False

            fill_q = deque()

            def weave(n):
                while n > 0 and fill_q:
                    u = fill_q[0]
                    if u.step():
                        n -= 1
                    else:
                        fill_q.popleft()

            def force(units):
                while not all(u.done for u in units):
                    weave(1)
                    if not fill_q:
                        break

            # ---- attention section for (head pair p, query block b) ----
            def norms(avs, p, b):
                # normalize by the broadcast fast-reciprocal of the ones-row
                # denominator; custom-DVE recip must not read PSUM: stage the
                # row in SBUF first.  The two subs' chains are interleaved so
                # the gpsimd broadcasts overlap the DVE ops.
                dn, rc, rb = [None, None], [None, None], [None, None]
                for s in (0, 1):
                    dn[s] = spool.tile([1, QB], f32, tag=f"dn{s}", name="dn")
                    nc.vector.tensor_copy(dn[s][:], avs[s][DH:DH + 1, :])
                    rc[s] = spool.tile([1, QB], f32, tag=f"rc{s}", name="rc")
                    nc.vector.reciprocal_approx_fast(rc[s][:], dn[s][:])
                for s in (0, 1):
                    rb[s] = rpool.tile([DH, QB], f32, tag=f"rb{s}", name="rb")
                    nc.gpsimd.partition_broadcast(rb[s][:], rc[s][:])
                for s in (0, 1):
                    nc.vector.tensor_mul(
                        aoT[p][DH * s:DH * s + DH, b * QB:(b + 1) * QB],
                        avs[s][0:DH, :], rb[s][:])

            def av_step(p, njt, avs, jt, et, c0):
                for s in (0, 1):
                    nc.tensor.matmul(
                        avs[s][:, c0:QB], vst[jt][:, 2 * p + s, :],
                        et[:, s, c0:QB],
                        start=(jt == 0), stop=(jt == njt - 1))

            WEAVE = (8, 6, 4, 4)   # filler steps per group, by block

            def section(p, b):
                njt = 4 * (b + 1)
                avs = [avp.tile([DH + 1, QB], f32, tag="av",
                                name=f"av{p}_{b}_{s}") for s in (0, 1)]
                prev = []
                for g in range(njt // 2):
                    # scores for both j-tiles of the group (64x128 tiling
                    # mode, pairs stream concurrently; 2nd pair's LDW hides
                    # behind the 1st pair's matmuls)
                    cur = []
                    for jt in (2 * g, 2 * g + 1):
                        off = jt - 4 * b
                        c0 = 128 * off if off > 0 else 0
                        ps = sp.tile([128, 2, QB], f32, tag="ps",
                                     name=f"ps{p}_{b}_{jt}")
                        for s in (0, 1):
                            nc.tensor.matmul(
                                ps[:, s, c0:QB],
                                kTt[p][DH * s:DH * s + DH,
                                       jt * 128:(jt + 1) * 128],
                                qT[p][DH * s:DH * s + DH,
                                      b * QB + c0:(b + 1) * QB],
                                start=True, stop=True)
                        cur.append((jt, ps, c0, off))
                    # previous group's AV pairs (128x128 mode)
                    for jt, et, c0 in prev:
                        av_step(p, njt, avs, jt, et, c0)
                    # exps (ScalarE) + diagonal masks (VectorE) for the group
                    nxt = []
                    for jt, ps, c0, off in cur:
                        et = epool.tile([128, 2, QB], bf16, tag="et",
                                        name="et")
                        nc.scalar.activation(et[:, :, c0:QB], ps[:, :, c0:QB],
                                             Exp, scale=SCALE)
                        if off >= 0:
                            for s in (0, 1):
                                nc.vector.tensor_mul(
                                    et[:, s, c0:c0 + 128],
                                    et[:, s, c0:c0 + 128], tri[:])
                        nxt.append((jt, et, c0))
                    weave(WEAVE[b])
                    prev = nxt
                for jt, et, c0 in prev:
                    av_step(p, njt, avs, jt, et, c0)
                norms(avs, p, b)

            # ---- schedule: 16 sections, deps forced at section start,
            # deps prefetched TWO sections ahead + out-projections woven ----
            def deps_for(i):
                b, p = divmod(i, 4)
                us = [Unit(qk_unit(p, b)), Unit(qk_unit(p + 4, b))]
                if p == 0:
                    us += [Unit(v_unit(st)) for st in range(4 * b, 4 * b + 4)]
                return us

            # STARTUP (DMA-paced): the first two sections' deps are emitted
            # round-robin per k-tile so every weight/x arrival has a runnable
            # matmul -- a FIFO weave would head-of-line-block on the slowest
            # stream.  Their accumulators borrow the (idle) scores/av PSUM
            # slots so all 8 units are in flight at once; warm-up matmuls
            # (dep-free) front-fill the first ~4us.
            s_ps0 = sp.tile([128, 2, QB], f32, tag="ps", name="s_ps0")
            s_ps1 = sp.tile([128, 2, QB], f32, tag="ps", name="s_ps1")
            s_av0 = avp.tile([128, HPG, DH], f32, tag="av", name="s_av0")
            s_av1 = avp.tile([128, HPG, DH], f32, tag="av", name="s_av1")
            s_pp0 = pp.tile([128, HPG, DH], f32, tag="pp", name="s_pp0")
            s_pp1 = pp.tile([128, HPG, DH], f32, tag="pp", name="s_pp1")
            start_units = [
                Unit(qk_unit(0, 0, s_ps0[:, 0, :])),
                Unit(qk_unit(4, 0, s_ps0[:, 1, :])),
                Unit(v_unit(0, s_av0)), Unit(v_unit(1, s_av1)),
                Unit(v_unit(2, s_pp0)), Unit(v_unit(3, s_pp1)),
                Unit(qk_unit(1, 0, s_ps1[:, 0, :])),
                Unit(qk_unit(5, 0, s_ps1[:, 1, :])),
            ]
            warm = Unit(warm_unit())
            while not warm.done:
                warm.step()
            while not all(u.done for u in start_units):
                for u in start_units:
                    u.step()

            U = {0: [], 1: []}
            tail_boxes = []
            pending_outs = deque()
            nxt_dep = 2
            for i in range(16):
                b, p = divmod(i, 4)
                horizon = 3 if i <= 4 else 2
                while nxt_dep < 16 and nxt_dep <= i + horizon:
                    U[nxt_dep] = deps_for(nxt_dep)
                    fill_q.extend(U[nxt_dep])
                    nxt_dep += 1
                if i == 15:
                    # last block: ALL eight (st, n) out-projection chunks
                    # pre-run k<=2 during the final section (evicting to
                    # SBUF so the two PSUM boxes recycle); only the k=3
                    # tails + adds + stores remain after the final norms.
                    # Prepended so they outrank leftover block-2 fillers.
                    for st in (15, 14, 13, 12):
                        for n in (1, 0):
                            box = []
                            tail_boxes.append((st, n, box))
                            fill_q.appendleft(Unit(out_head(st, n, box)))
                    tail_boxes.reverse()
                # drip the previous block's out-projections (3 per section)
                # so their VectorE evictions don't burst at block boundaries
                for _ in range(3):
                    if pending_outs:
                        fill_q.append(Unit(out_unit(*pending_outs.popleft())))
                force(U[i])
                section(p, b)
                if p == 3 and b < 3:
                    pending_outs.extend(
                        (st, n) for st in range(4 * b, 4 * b + 4)
                        for n in range(2))
            for st, n in pending_outs:
                fill_q.append(Unit(out_unit(st, n)))
            for st, n, box in tail_boxes:
                fill_q.append(Unit(out_tail(st, n, box)))
            while fill_q:
                weave(1)

    nc.compile()
    return nc


def _shard_inputs(x, w_qkv, b_qkv, w_out):
    # keep key j (partition) <= query i (free column): upper triangle
    tri = np.triu(np.ones((128, 128))).astype(BF16)
    in_maps = []
    for c in range(N_CORES):
        b, g = c // G, c % G
        sl = slice(g * CG, (g + 1) * CG)
        bq = b_qkv[0 * D:1 * D][sl].reshape(CG // 128, 128).T
        bk = b_qkv[1 * D:2 * D][sl].reshape(CG // 128, 128).T
        in_maps.append({
            "xT": np.ascontiguousarray(x[b].T).astype(BF16),
            "wq": w_qkv[:, 0 * D:1 * D][:, sl].astype(BF16),
            "wk": w_qkv[:, 1 * D:2 * D][:, sl].astype(BF16),
            "wv": w_qkv[:, 2 * D:3 * D][:, sl].astype(BF16),
            "bqk": np.ascontiguousarray(
                np.concatenate([bq, bk], axis=1)).astype(np.float32),
            "wo": w_out[sl, :].astype(BF16),
            "tri": tri,
        })
    return in_maps


def kernel(x, w_qkv, b_qkv, w_out, b_out):
    from concourse.bass_utils import run_bass_kernel_spmd

    x = np.asarray(x, np.float32)
    w_qkv = np.asarray(w_qkv, np.float32)
    b_qkv = np.asarray(b_qkv, np.float32)
    w_out = np.asarray(w_out, np.float32)
    b_out = np.asarray(b_out, np.float32)

    if "nc" not in _cache:
        _cache["nc"] = _build_program()
    nc = _cache["nc"]

    in_maps = _shard_inputs(x, w_qkv, b_qkv, w_out)
    res = run_bass_kernel_spmd(nc, in_maps, core_ids=list(range(N_CORES)))
    _cache["last_result"] = res

    # v-projection bias contributes bv @ w_out (a constant row) to y:
    # attention weights sum to 1, so it survives softmax-averaging intact.
    bias = b_out + np.asarray(b_qkv[2 * D:3 * D], np.float32) @ w_out
    y = np.empty((B, S, D), np.float32)
    for b in range(B):
        y[b] = (res.results[G * b]["y"].astype(np.float32)
                + res.results[G * b + 1]["y"].astype(np.float32) + bias)
    return y


# revision 37
# speedup vs baseline: 1.0184x; 1.0010x over previous
"""Trainium2 Bass kernel for causal multi-head attention (software-pipelined).

Problem: B=4, S=2048, D=1024, H=16 heads (d_head=64), fp32 I/O.
    qkv = x @ w_qkv + b_qkv ; causal softmax attention ; out @ w_out + b_out

Sharding over 8 NeuronCores: data-parallel over batch (4) x tensor-parallel
over head-groups (2 groups of 8 heads). Core c handles batch c//2, head-group
c%2. No collectives: each core returns its partial out-projection
y_partial = attn_out_g @ w_out[rows_g]; the host sums the two group partials
per batch and adds b_out (plus b_v @ w_out -- see below).

Design notes (v2, evolved from the 311us phase-woven baseline):
  - 512-wide query blocks b=0..3; key j-tiles of 128; head pairs p=0..3
    (heads 2p, 2p+1 live in partition halves of qT/kT tiles; the two scores
    matmuls per j-tile are 64x128 row-tiles T0/T8 that stream CONCURRENTLY).
  - j-tiles processed in GROUPS OF TWO per PE mode: [scores jt0 + jt1]
    (64x128 tiling mode) then [AV jt0' + jt1' of the previous group]
    (128x128 mode).  Mode switches drain the PE array (~110ns each);
    grouping halves them vs per-j-tile alternation (trace: scores pair
    317ns vs 216 roofline = unhidden LDW + drain; AV s0 336 vs 216).
  - The two packed heads write ONE merged scores PSUM tile [128, 2, 512]
    (2 banks), one ScalarE exp covers both heads per j-tile.
  - AV accumulates [65, 512] per sub with a ones column in v producing the
    softmax denominator; max-free softmax (logits < ~7).
  - v-projection bias is folded OUT of the device: sum(attn)=1 makes
    av_norm(v_raw + bv) = av_norm(v_raw) + bv, and bv passes through the
    out-projection as the constant row bv @ w_out added host-side into
    b_out.  This deletes 16 K=1 N=512 bias matmuls (~5us PE).
  - Projections (qk via stationary w, v natural) and the out-projection are
    split into small units woven between attention groups from a generator
    queue; deps are prefetched TWO sections ahead so block transitions
    (2 qk + 4 v units = ~13us of PE) never starve the weave.
  - qk bias+PSUM-drain runs on VectorE (tensor_scalar_add), ScalarE stays
    exp-only.
  - PSUM budget: scores 2x2 + av 2x1 + proj/out 2x1 = 8 banks exactly.
  - DMA: inputs split across both HWDGE queues (Sync: wq + x[:, :512] the
    lead needs, then x tails + wo; Scalar: small consts + wk + wv).
    bq/bk packed host-side into one [128, 8] tensor.  A dummy exp preloads
    the ACT table during the DMA lead; a dependency-free 36-matmul warm-up
    block on a MEMSET tile (no DMA gate) un-throttles the HAM clock gate
    ~7us earlier than the tri-gated variant.
  - Output y is stored bf16 (halves the 8MB/core store drain; host sums
    the two partials in f32; adds ~0.1% rounding, budget is 2e-2).
  - Tail: the last block's first two out-projection chunks split into a
    k<=2 partial woven during the final section + a k=3 tail after the
    final norms (accumulation-group semaphore waits hoist to the group
    head and would otherwise serialize behind the final normalization).
"""

import sys

if "/opt/trn_rl_repo" not in sys.path:
    sys.path.insert(0, "/opt/trn_rl_repo")

from collections import deque

import numpy as np
import ml_dtypes

B, S, D = 4, 2048, 1024
H, DH = 16, 64
G = 2                # tensor-parallel head groups
HPG = H // G         # heads per group (8)
CG = HPG * DH        # channel cols per group (512)
N_CORES = 8
BF16 = ml_dtypes.bfloat16

KT = D // 128        # 8 contraction k-tiles for the projections
QB = 512             # query block width
NB = S // QB         # 4 query blocks

_cache = {}


def _build_program():
    import concourse.tile as tile
    from concourse import bacc, mybir

    f32 = mybir.dt.float32
    bf16 = mybir.dt.bfloat16
    Exp = mybir.ActivationFunctionType.Exp
    Copy = mybir.ActivationFunctionType.Copy
    SCALE = float(DH) ** -0.5

    nc = bacc.Bacc("TRN2", target_bir_lowering=False, debug=False,
                   num_devices=N_CORES)

    xT_d = nc.dram_tensor("xT", [D, S], bf16, kind="ExternalInput").ap()
    wq_d = nc.dram_tensor("wq", [D, CG], bf16, kind="ExternalInput").ap()
    wk_d = nc.dram_tensor("wk", [D, CG], bf16, kind="ExternalInput").ap()
    wv_d = nc.dram_tensor("wv", [D, CG], bf16, kind="ExternalInput").ap()
    # bq/bk packed as one [128, 8] tensor (col m = bq chunk m, col 4+m = bk
    # chunk m): a single DMA instead of eight 4-byte-element ones
    bqk_d = nc.dram_tensor("bqk", [128, 8], f32, kind="ExternalInput").ap()
    wo_d = nc.dram_tensor("wo", [CG, D], bf16, kind="ExternalInput").ap()
    tri_d = nc.dram_tensor("tri", [128, 128], bf16, kind="ExternalInput").ap()
    y_d = nc.dram_tensor("y", [S, D], bf16, kind="ExternalOutput").ap()

    with tile.TileContext(nc) as tc:
        with (
            tc.tile_pool(name="consts", bufs=1) as cpool,
            tc.tile_pool(name="acts", bufs=1) as apool,
            tc.tile_pool(name="exps", bufs=6) as epool,
            tc.tile_pool(name="small", bufs=2) as spool,
            tc.tile_pool(name="rbc", bufs=2) as rpool,
            tc.tile_pool(name="ystage", bufs=3) as ypool,
            tc.tile_pool(name="psum_s", bufs=2, space="PSUM") as sp,
            tc.tile_pool(name="psum_av", bufs=2, space="PSUM") as avp,
            tc.tile_pool(name="psum_p", bufs=2, space="PSUM") as pp,
        ):
            # ---- DMA loads, split across the two HWDGE queues (Sync +
            # Scalar) to double input bandwidth. ----
            ones_row = cpool.tile([1, 128], bf16, tag="ones")
            nc.gpsimd.memset(ones_row[:], 1.0)
            # warm-up stationary: memset (NOT DMA-gated) so the HAM warm-up
            # can start during the bass preamble
            wrm = cpool.tile([128, 128], bf16, tag="wrm")
            nc.gpsimd.memset(wrm[:], 0.25)
            # dummy exp on a const tile: forces the ACT table load (~2.7us)
            # to happen during the DMA lead, before the ScalarE-queue DMAs
            warm_exp = spool.tile([1, 128], f32, tag="wexp", name="warm_exp")
            nc.scalar.activation(warm_exp[:], ones_row[:], Exp, scale=1.0)

            bqk = cpool.tile([128, 8], f32, tag="bqk")
            nc.scalar.dma_start(bqk[:], bqk_d[:])
            bqc = [bqk[:, m:m + 1] for m in range(4)]
            bkc = [bqk[:, 4 + m:5 + m] for m in range(4)]
            tri = cpool.tile([128, 128], bf16, tag="tri")
            nc.scalar.dma_start(tri[:], tri_d[:])

            wq, wk, wv, xt = [], [], [], []
            for k in range(KT):
                rows = slice(k * 128, (k + 1) * 128)
                t = cpool.tile([128, CG], bf16, tag=f"wq{k}", name=f"wq{k}")
                nc.sync.dma_start(t[:], wq_d[rows, :])
                wq.append(t)
                t = cpool.tile([128, CG], bf16, tag=f"wk{k}", name=f"wk{k}")
                nc.scalar.dma_start(t[:], wk_d[rows, :])
                wk.append(t)
                t = cpool.tile([128, CG], bf16, tag=f"wv{k}", name=f"wv{k}")
                nc.scalar.dma_start(t[:], wv_d[rows, :])
                wv.append(t)
                t = cpool.tile([128, S], bf16, tag=f"xt{k}", name=f"xt{k}")
                nc.sync.dma_start(t[:, 0:QB], xT_d[rows, 0:QB])
                xt.append(t)
            for k in range(KT):
                nc.sync.dma_start(xt[k][:, QB:S],
                                  xT_d[k * 128:(k + 1) * 128, QB:S])
            wo = []
            for k in range(CG // 128):
                t = cpool.tile([128, D], bf16, tag=f"wo{k}", name=f"wo{k}")
                nc.sync.dma_start(t[:], wo_d[k * 128:(k + 1) * 128, :])
                wo.append(t)

            # ---- persistent activations ----
            qT = [apool.tile([128, S], bf16, tag=f"qT{m}", name=f"qT{m}")
                  for m in range(CG // 128)]
            kTt = [apool.tile([128, S], bf16, tag=f"kT{m}", name=f"kT{m}")
                   for m in range(CG // 128)]
            # v with a ones column per head: [s, h, 0:64] = v_h, [s, h, 64] = 1
            vst = [apool.tile([128, HPG, DH + 1], bf16, tag=f"v{st}",
                              name=f"v{st}")
                   for st in range(S // 128)]
            aoT = [apool.tile([128, S], bf16, tag=f"aoT{m}", name=f"aoT{m}")
                   for m in range(CG // 128)]

            for st in range(S // 128):
                nc.gpsimd.memset(vst[st][:, :, DH:DH + 1], 1.0)

            # ---- filler units (generators yielding between PE matmuls) ----
            def qk_unit(m, n, acc=None):
                wt, bc, dst = (wq, bqc, qT) if m < 4 else (wk, bkc, kTt)
                mi = m % 4
                if acc is None:
                    acc = pp.tile([128, QB], f32, tag="pp", name=f"qk{m}_{n}")
                for k in range(KT):
                    nc.tensor.matmul(
                        acc[:], wt[k][:, mi * 128:(mi + 1) * 128],
                        xt[k][:, n * QB:(n + 1) * QB],
                        start=(k == 0), stop=(k == KT - 1))
                    yield
                nc.vector.tensor_scalar_add(
                    dst[mi][:, n * QB:(n + 1) * QB], acc[:], bc[mi])
                yield

            def v_unit(st, acc=None):
                if acc is None:
                    acc = pp.tile([128, HPG, DH], f32, tag="pp",
                                  name=f"vacc{st}")
                for k in range(KT):
                    nc.tensor.matmul(
                        acc[:, :, :], xt[k][:, st * 128:(st + 1) * 128],
                        wv[k][:], start=(k == 0), stop=(k == KT - 1))
                    yield
                nc.vector.tensor_copy(vst[st][:, :, 0:DH], acc[:, :, :])
                yield

            def out_unit(st, n):
                yp = pp.tile([128, 512], f32, tag="pp", name=f"yp{st}_{n}")
                for k in range(CG // 128):
                    nc.tensor.matmul(
                        yp[:], aoT[k][:, st * 128:(st + 1) * 128],
                        wo[k][:, n * 512:(n + 1) * 512],
                        start=(k == 0), stop=(k == CG // 128 - 1))
                    yield
                ys = ypool.tile([128, 512], bf16, tag="ys", name=f"ys{st}_{n}")
                nc.vector.tensor_copy(ys[:], yp[:])
                nc.sync.dma_start(
                    y_d[st * 128:(st + 1) * 128, n * 512:(n + 1) * 512], ys[:])
                yield

            # split variants for the LAST block: the k=0..2 partial (deps:
            # head pairs 0..2 only) weaves during the final section and is
            # EVICTED to SBUF bf16, freeing its PSUM box so all 8 chunks can
            # pre-run; the k=3 matmul + add + store runs after the final
            # norm.  Costs one extra bf16 rounding on 3/4 of y (~0.1%).
            def out_head(st, n, box):
                yp = pp.tile([128, 512], f32, tag="pp", name=f"yph{st}_{n}")
                for k in range(3):
                    nc.tensor.matmul(
                        yp[:], aoT[k][:, st * 128:(st + 1) * 128],
                        wo[k][:, n * 512:(n + 1) * 512],
                        start=(k == 0), stop=(k == 2))
                    yield
                part = ypool.tile([128, 512], bf16, tag=f"part{st}_{n}",
                                  bufs=1, name=f"part{st}_{n}")
                box.append(part)
                nc.vector.tensor_copy(part[:], yp[:])
                yield

            def out_tail(st, n, box):
                yp = pp.tile([128, 512], f32, tag="pp", name=f"ypt{st}_{n}")
                nc.tensor.matmul(
                    yp[:], aoT[3][:, st * 128:(st + 1) * 128],
                    wo[3][:, n * 512:(n + 1) * 512],
                    start=True, stop=True)
                yield
                ys = ypool.tile([128, 512], bf16, tag="ys", name=f"ys{st}_{n}")
                nc.vector.tensor_add(ys[:], yp[:], box[0][:])
                nc.sync.dma_start(
                    y_d[st * 128:(st + 1) * 128, n * 512:(n + 1) * 512], ys[:])
                yield

            def warm_unit():
                # ~4us of dependency-free full-array matmuls: fills the
                # preamble/DMA lead and un-throttles the HAM clock gate
                # (which watches PE array activity) before dense work.
                wp = pp.tile([128, 128], f32, tag="pp", name="warmps")
                for i in range(36):
                    nc.tensor.matmul(wp[:], wrm[:], wrm[:],
                                     start=True, stop=True)
                    yield

            class Unit:
                __slots__ = ("gen", "done")

                def __init__(self, gen):
                    self.gen = gen
                    self.done = False

                def step(self):
                    if self.done:
                        return False
                    try:
                        next(self.gen)
                        return True
                    except StopIteration:
                        self.done = True
                        return False

            fill_q = deque()

            def weave(n):
                while n > 0 and fill_q:
                    u = fill_q[0]
                    if u.step():
                        n -= 1
                    else:
                        fill_q.popleft()

            def force(units):
                while not all(u.done for u in units):
                    weave(1)
                    if not fill_q:
                        break

            # ---- attention section for (head pair p, query block b) ----
            def norms(avs, p, b):
                # normalize by the broadcast fast-reciprocal of the ones-row
                # denominator; custom-DVE recip must not read PSUM: stage the
                # row in SBUF first.  The two subs' chains are interleaved so
                # the gpsimd broadcasts overlap the DVE ops.
                dn, rc, rb = [None, None], [None, None], [None, None]
                for s in (0, 1):
                    dn[s] = spool.tile([1, QB], f32, tag=f"dn{s}", name="dn")
                    rc[s] = spool.tile([1, QB], f32, tag=f"rc{s}", name="rc")
                if p == 3 and b == 3:
                    # final section: exps are done, ScalarE is idle -- stage
                    # the two denominators in parallel across engines (s=1's
                    # chain fully on VectorE first so it never waits s=0's
                    # ScalarE copy)
                    nc.scalar.activation(dn[0][:], avs[0][DH:DH + 1, :],
                                         Copy, scale=1.0)
                    nc.vector.tensor_copy(dn[1][:], avs[1][DH:DH + 1, :])
                    nc.vector.reciprocal_approx_fast(rc[1][:], dn[1][:])
                    nc.vector.reciprocal_approx_fast(rc[0][:], dn[0][:])
                else:
                    for s in (0, 1):
                        nc.vector.tensor_copy(dn[s][:], avs[s][DH:DH + 1, :])
                        nc.vector.reciprocal_approx_fast(rc[s][:], dn[s][:])
                for s in (0, 1):
                    rb[s] = rpool.tile([DH, QB], f32, tag=f"rb{s}", name="rb")
                    nc.gpsimd.partition_broadcast(rb[s][:], rc[s][:])
                for s in (0, 1):
                    nc.vector.tensor_mul(
                        aoT[p][DH * s:DH * s + DH, b * QB:(b + 1) * QB],
                        avs[s][0:DH, :], rb[s][:])

            def av_step(p, njt, avs, jt, et, c0):
                for s in (0, 1):
                    nc.tensor.matmul(
                        avs[s][:, c0:QB], vst[jt][:, 2 * p + s, :],
                        et[:, s, c0:QB],
                        start=(jt == 0), stop=(jt == njt - 1))

            WEAVE = (8, 6, 4, 4)   # filler steps per group, by block

            def section(p, b):
                njt = 4 * (b + 1)
                avs = [avp.tile([DH + 1, QB], f32, tag="av",
                                name=f"av{p}_{b}_{s}") for s in (0, 1)]
                prev = []
                for g in range(njt // 2):
                    # scores for both j-tiles of the group (64x128 tiling
                    # mode, pairs stream concurrently; 2nd pair's LDW hides
                    # behind the 1st pair's matmuls)
                    cur = []
                    for jt in (2 * g, 2 * g + 1):
                        off = jt - 4 * b
                        c0 = 128 * off if off > 0 else 0
                        ps = sp.tile([128, 2, QB], f32, tag="ps",
                                     name=f"ps{p}_{b}_{jt}")
                        for s in (0, 1):
                            nc.tensor.matmul(
                                ps[:, s, c0:QB],
                                kTt[p][DH * s:DH * s + DH,
                                       jt * 128:(jt + 1) * 128],
                                qT[p][DH * s:DH * s + DH,
                                      b * QB + c0:(b + 1) * QB],
                                start=True, stop=True)
                        cur.append((jt, ps, c0, off))
                    # previous group's AV pairs (128x128 mode)
                    for jt, et, c0 in prev:
                        av_step(p, njt, avs, jt, et, c0)
                    # exps (ScalarE) + diagonal masks (VectorE) for the group
                    nxt = []
                    for jt, ps, c0, off in cur:
                        et = epool.tile([128, 2, QB], bf16, tag="et",
                                        name="et")
                        nc.scalar.activation(et[:, :, c0:QB], ps[:, :, c0:QB],
                                             Exp, scale=SCALE)
                        if off >= 0:
                            for s in (0, 1):
                                nc.vector.tensor_mul(
                                    et[:, s, c0:c0 + 128],
                                    et[:, s, c0:c0 + 128], tri[:])
                        nxt.append((jt, et, c0))
                    weave(WEAVE[b])
                    prev = nxt
                for jt, et, c0 in prev:
                    av_step(p, njt, avs, jt, et, c0)
                norms(avs, p, b)

            # ---- schedule: 16 sections, deps forced at section start,
            # deps prefetched TWO sections ahead + out-projections woven ----
            def deps_for(i):
                b, p = divmod(i, 4)
                us = [Unit(qk_unit(p, b)), Unit(qk_unit(p + 4, b))]
                if p == 0:
                    us += [Unit(v_unit(st)) for st in range(4 * b, 4 * b + 4)]
                return us

            # STARTUP (DMA-paced): the first two sections' deps are emitted
            # round-robin per k-tile so every weight/x arrival has a runnable
            # matmul -- a FIFO weave would head-of-line-block on the slowest
            # stream.  Their accumulators borrow the (idle) scores/av PSUM
            # slots so all 8 units are in flight at once; warm-up matmuls
            # (dep-free) front-fill the first ~4us.
            s_ps0 = sp.tile([128, 2, QB], f32, tag="ps", name="s_ps0")
            s_ps1 = sp.tile([128, 2, QB], f32, tag="ps", name="s_ps1")
            s_av0 = avp.tile([128, HPG, DH], f32, tag="av", name="s_av0")
            s_av1 = avp.tile([128, HPG, DH], f32, tag="av", name="s_av1")
            s_pp0 = pp.tile([128, HPG, DH], f32, tag="pp", name="s_pp0")
            s_pp1 = pp.tile([128, HPG, DH], f32, tag="pp", name="s_pp1")
            start_units = [
                Unit(qk_unit(0, 0, s_ps0[:, 0, :])),
                Unit(qk_unit(4, 0, s_ps0[:, 1, :])),
                Unit(v_unit(0, s_av0)), Unit(v_unit(1, s_av1)),
                Unit(v_unit(2, s_pp0)), Unit(v_unit(3, s_pp1)),
                Unit(qk_unit(1, 0, s_ps1[:, 0, :])),
                Unit(qk_unit(5, 0, s_ps1[:, 1, :])),
            ]
            warm = Unit(warm_unit())
            while not warm.done:
                warm.step()
            while not all(u.done for u in start_units):
                for u in start_units:
                    u.step()

            U = {0: [], 1: []}
            tail_boxes = []
            pending_outs = deque()
            nxt_dep = 2
            for i in range(16):
                b, p = divmod(i, 4)
                horizon = 3 if i <= 4 else 2
                while nxt_dep < 16 and nxt_dep <= i + horizon:
                    U[nxt_dep] = deps_for(nxt_dep)
                    fill_q.extend(U[nxt_dep])
                    nxt_dep += 1
                if i == 15:
                    # last block: ALL eight (st, n) out-projection chunks
                    # pre-run k<=2 during the final section (evicting to
                    # SBUF so the two PSUM boxes recycle); only the k=3
                    # tails + adds + stores remain after the final norms.
                    # Prepended so they outrank leftover block-2 fillers.
                    for st in (15, 14, 13, 12):
                        for n in (1, 0):
                            box = []
                            tail_boxes.append((st, n, box))
                            fill_q.appendleft(Unit(out_head(st, n, box)))
                    tail_boxes.reverse()
                # drip the previous block's out-projections (3 per section)
                # so their VectorE evictions don't burst at block boundaries
                for _ in range(3):
                    if pending_outs:
                        fill_q.append(Unit(out_unit(*pending_outs.popleft())))
                force(U[i])
                section(p, b)
                if p == 3 and b < 3:
                    pending_outs.extend(
                        (st, n) for st in range(4 * b, 4 * b + 4)
                        for n in range(2))
            for st, n in pending_outs:
                fill_q.append(Unit(out_unit(st, n)))
            for st, n, box in tail_boxes:
                fill_q.append(Unit(out_tail(st, n, box)))
            while fill_q:
                weave(1)

    nc.compile()
    return nc


def _shard_inputs(x, w_qkv, b_qkv, w_out):
    # keep key j (partition) <= query i (free column): upper triangle
    tri = np.triu(np.ones((128, 128))).astype(BF16)
    in_maps = []
    for c in range(N_CORES):
        b, g = c // G, c % G
        sl = slice(g * CG, (g + 1) * CG)
        bq = b_qkv[0 * D:1 * D][sl].reshape(CG // 128, 128).T
        bk = b_qkv[1 * D:2 * D][sl].reshape(CG // 128, 128).T
        in_maps.append({
            "xT": np.ascontiguousarray(x[b].T).astype(BF16),
            "wq": w_qkv[:, 0 * D:1 * D][:, sl].astype(BF16),
            "wk": w_qkv[:, 1 * D:2 * D][:, sl].astype(BF16),
            "wv": w_qkv[:, 2 * D:3 * D][:, sl].astype(BF16),
            "bqk": np.ascontiguousarray(
                np.concatenate([bq, bk], axis=1)).astype(np.float32),
            "wo": w_out[sl, :].astype(BF16),
            "tri": tri,
        })
    return in_maps


def kernel(x, w_qkv, b_qkv, w_out, b_out):
    from concourse.bass_utils import run_bass_kernel_spmd

    x = np.asarray(x, np.float32)
    w_qkv = np.asarray(w_qkv, np.float32)
    b_qkv = np.asarray(b_qkv, np.float32)
    w_out = np.asarray(w_out, np.float32)
    b_out = np.asarray(b_out, np.float32)

    if "nc" not in _cache:
        _cache["nc"] = _build_program()
    nc = _cache["nc"]

    in_maps = _shard_inputs(x, w_qkv, b_qkv, w_out)
    res = run_bass_kernel_spmd(nc, in_maps, core_ids=list(range(N_CORES)))
    _cache["last_result"] = res

    # v-projection bias contributes bv @ w_out (a constant row) to y:
    # attention weights sum to 1, so it survives softmax-averaging intact.
    bias = b_out + np.asarray(b_qkv[2 * D:3 * D], np.float32) @ w_out
    y = np.empty((B, S, D), np.float32)
    for b in range(B):
        y[b] = (res.results[G * b]["y"].astype(np.float32)
                + res.results[G * b + 1]["y"].astype(np.float32) + bias)
    return y
